# revision 47
# baseline (speedup 1.0000x reference)
"""Trainium2 Bass kernel for nn_Attention_layer_12249246728743.

Reference structure (after untangling the C-order reshape): per channel c
of 512, the 3136 raster positions split into 49 segments of 64
consecutive positions; each segment attends over a 7x7 shifted window of
its OWN channel plane (depthwise local attention):

  scores[c,s,p=(i,j)] = sum_d q[c,64s+d] * k[c, win(64s+d, i, j)]
                        + (sum_d q[c,64s+d]) * bias49[p]
  w = softmax_p(scores);  out[c,64s+d] = sum_p w[c,s,p] * v[c, win(...)]

with q/k/v = 1x1 convs of x (k, v on the zero-padded 62x62 domain).

Sharding: channel-parallel across 8 cores (64 channels each); every
attention segment is core-local: no halo, no collectives.

Layout: "pair-packed" attention - partitions = 64 channels x 2
shift-pair halves, free dim = the full 3136-position raster. The B half
holds k/v planes pre-shifted by +1 (or +56 for row-wrapping pairs), so
ONE tensor op computes two of the 49 window shifts at once (25 ops
instead of 49, no half-image padding waste). QK segment sums use
contiguous-half bf16 add trees (2x DVE mode) + a small TensorReduce
instead of full-width fp32 reduces. AV weights enter the multiply as
stride-0 broadcast APs (free-size-matched zip with the window AP), and
the 25 AV product tiles are accumulated on the otherwise-idle PE via
identity matmuls into PSUM. qsum (rank-1 bias term) is exact host-side
algebra (q_w @ segment-sums-of-x), mirroring the host-collapsed bias49.
Work is split between DVE and GPSIMD by a greedy makespan balancer.
"""

import numpy as np

import concourse.bass as bass
import concourse.mybir as mybir
import concourse.tile as tile
from concourse.bass_utils import run_bass_kernel_spmd

F32 = mybir.dt.float32
BF16 = mybir.dt.bfloat16
AX = mybir.AxisListType
OP = mybir.AluOpType
AF = mybir.ActivationFunctionType

N_CORES = 8
C = 512
H = W = 56
HP = WP = 62          # padded spatial
NPOS = H * W          # 3136
NPAD = HP * WP        # 3844
K = 7
NSH = K * K           # 49 shifts
SEG = 64              # positions per attention segment
NSEG = NPOS // SEG    # 49 segments per channel
CH = 64               # channels per core
NPAIR = 25            # 24 shift pairs + 1 single (p=48)

# shift pairs (pA=2t, pB=2t+1): B half of the k/v tiles is pre-shifted by
# +1 (same-row j->j+1) or +56 (row wrap (i,6)->(i+1,0)). Window slice is
# always pA's (i,j) clamped to j<=6.
def _pair_table():
    pairs = []
    for t in range(24):
        pA = 2 * t
        i, j = divmod(pA, K)
        if j < K - 1:
            pairs.append((i, j, "k1"))     # B = (i, j+1) via +1 tile
        else:
            pairs.append((i, j, "k56"))    # B = (i+1, 0) via +56 tile
    pairs.append((6, 6, "k1"))             # p=48 single; B half masked
    return pairs


PAIRS = _pair_table()

# --- naive op-cost mirror (documented TRN2 formulas) for the balancer ---
def _c_tt(fd, bf16_out=True):
    return (58 + (fd / 2 if bf16_out else fd)) / 0.96


def _c_red(fd_in):
    return (58 + fd_in) / 0.96


class _Sched:
    """DVE/GPSIMD makespan balancer (Pool runs at 2x cost).

    Costs are static, so assignment is planned offline: LPT over the
    descending-sorted schedulable costs against the DVE-pinned base load,
    consumed in emission order by pick().
    """

    def __init__(self, nc, pe_base):
        self.nc = nc
        self.v = 0.0
        self.g = 0.0
        self.pe = pe_base
        self.plan = []

    def make_plan(self, costs, pinned_v):
        v, g = pinned_v, 0.0
        asg = [None] * len(costs)
        for i in sorted(range(len(costs)), key=lambda i: -costs[i]):
            c = costs[i]
            if max(v + c, 2 * g) <= max(v, 2 * (g + c)):
                v += c
                asg[i] = "v"
            else:
                g += c
                asg[i] = "g"
        self.plan = asg[::-1]          # consume via pop()

    def pick(self, cost):
        e = self.plan.pop() if self.plan else (
            "v" if max(self.v + cost, self.g) <= max(self.v,
                                                     self.g + 2 * cost)
            else "g")
        if e == "v":
            self.v += cost
            return self.nc.vector
        self.g += 2 * cost
        return self.nc.gpsimd




def _build_nc():
    nc = bass.Bass()

    xp = nc.declare_dram_parameter("xp", [C, NPAD], F32, isOutput=False)
    wT = nc.declare_dram_parameter("wT", [C, 3 * CH], F32, isOutput=False)
    bqk = nc.declare_dram_parameter("bqk", [2 * CH, 1], F32, isOutput=False)
    bv = nc.declare_dram_parameter("bv", [CH, 1], F32, isOutput=False)
    bk0 = nc.declare_dram_parameter("bk0", [CH, 1], F32, isOutput=False)
    b49c = nc.declare_dram_parameter("b49c", [128, NPAIR], F32,
                                     isOutput=False)
    qsumd = nc.declare_dram_parameter("qsumd", [128, NSEG], F32,
                                      isOutput=False)
    identp = nc.declare_dram_parameter("identp", [128, 64], F32,
                                       isOutput=False)
    out_d = nc.declare_dram_parameter("out", [CH, NPOS], F32, isOutput=True)

    RPC = 8                      # rows per conv chunk
    NCH = RPC * WP               # 496 conv cols per chunk
    NCHI = RPC * W               # 448 interior positions per chunk
    NCK = 7                      # conv chunks (7*8 = 56 rows)
    AVC = 448                    # AV psum chunk (7 segments)
    NAV = 7                      # AV chunks

    with tile.TileContext(nc) as tc:
        with (
            tc.tile_pool(name="persist", bufs=1) as pp,
            tc.tile_pool(name="work", bufs=2) as wp,
            tc.tile_pool(name="psum", bufs=1, space="PSUM") as psp,
        ):
            MM = lambda n: max(60, 6 + n) / 2.4 + 128 / 1.2
            PE_BASE = 56 * MM(496) + MM(1) + 25 * 7 * MM(448)
            sch = _Sched(nc, PE_BASE)

            # ---- loads ----
            xt_all = pp.tile([128, 4 * NPAD], F32, tag="xall", name="xall")
            wt_all = pp.tile([128, 4 * 3 * CH], F32, tag="wall", name="wall")
            for s0 in range(0, NPAD, 1024):
                sn = min(1024, NPAD - s0)
                nc.sync.dma_start(
                    xt_all[:].rearrange("p (k n) -> p k n", k=4)[:, :, s0:s0 + sn],
                    xp[:].rearrange("(k p) n -> p k n", p=128)[:, :, s0:s0 + sn])
            nc.sync.dma_start(
                wt_all[:].rearrange("p (k n) -> p k n", k=4),
                wT[:].rearrange("(k p) n -> p k n", p=128))
            xt = [xt_all[:].rearrange("p (k n) -> p k n", k=4)[:, kt, :]
                  for kt in range(4)]
            wt = [wt_all[:].rearrange("p (k n) -> p k n", k=4)[:, kt, :]
                  for kt in range(4)]
            bqk_s = pp.tile([128, 1], F32, tag="bqk", name="bqk")
            bv_s = pp.tile([CH, 1], F32, tag="bv", name="bv")
            bk0_s = pp.tile([CH, 1], F32, tag="bk0", name="bk0")
            nc.sync.dma_start(bk0_s[:], bk0[:])
            b49_s = pp.tile([128, NPAIR], F32, tag="b49", name="b49")
            qsum_s = pp.tile([128, NSEG], F32, tag="qsum", name="qsum")
            idf_s = pp.tile([128, 64], F32, tag="idf", name="idf")
            nc.sync.dma_start(bqk_s[:], bqk[:])
            nc.sync.dma_start(bv_s[:], bv[:])
            nc.sync.dma_start(b49_s[:], b49c[:])
            nc.sync.dma_start(qsum_s[:], qsumd[:])
            nc.sync.dma_start(idf_s[:], identp[:])
            ident = idf_s[:].bitcast(BF16)          # [128, 128] bf16 identity

            # ---- attention-layout staging tiles ----
            qb = pp.tile([128, NPOS], BF16, tag="qb", name="qb")
            k1 = pp.tile([128, NPAD], BF16, tag="k1", name="k1")
            k56 = pp.tile([128, NPAD], BF16, tag="k56", name="k56")
            v1 = pp.tile([128, NPAD], BF16, tag="v1", name="v1")
            v56 = pp.tile([128, NPAD], BF16, tag="v56", name="v56")

            # conv / QK-tree-l1 / AV shared PSUM accumulators (8 banks)
            psa = [psp.tile([128, 512], F32, tag=f"psa{j}", name=f"psa{j}")
                   for j in range(8)]

            # PE pre-touch (keeps every real Matmult at <=1 sem wait for
            # walrus S3_LW codegen).
            nc.tensor.matmul(psa[7][0:1, 0:1], lhsT=xt_all[0:1, 0:1],
                             rhs=xt_all[0:1, 0:1], start=True, stop=True)

            # ---- 1x1 convs on interior rows (rows 3..58 of padded) ----
            for ci in range(NCK):
                r0 = 3 + RPC * ci
                pq = psa[(2 * ci) % 6]
                pv = psa[(2 * ci) % 6 + 1]
                for kt in range(4):
                    rhs = xt[kt][:, WP * r0: WP * r0 + NCH]
                    nc.tensor.matmul(pq[:, 0:NCH], lhsT=wt[kt][:, 0:128],
                                     rhs=rhs, start=(kt == 0), stop=(kt == 3))
                    nc.tensor.matmul(pv[0:CH, 0:NCH],
                                     lhsT=wt[kt][:, 128:192],
                                     rhs=rhs, start=(kt == 0), stop=(kt == 3))
                pq3 = pq[:, 0:NCH].rearrange("a (r c) -> a r c", c=WP)
                pv3 = pv[0:CH, 0:NCH].rearrange("a (r c) -> a r c", c=WP)
                qbv = qb[:, NCHI * ci: NCHI * (ci + 1)].rearrange(
                    "a (r c) -> a r c", c=W)
                nc.scalar.activation(qbv[0:CH], pq3[0:CH, :, 3:59],
                                     AF.Identity, bias=bqk_s[0:CH, :])
                nc.scalar.activation(qbv[CH:128], pq3[0:CH, :, 3:59],
                                     AF.Identity, bias=bqk_s[0:CH, :])
                k13 = k1[0:CH, :].rearrange("a (r c) -> a r c", c=WP)
                v13 = v1[0:CH, :].rearrange("a (r c) -> a r c", c=WP)
                nc.scalar.activation(k13[:, r0:r0 + RPC, 3:59],
                                     pq3[CH:128, :, 3:59], AF.Identity,
                                     bias=bqk_s[CH:128, :])
                nc.scalar.activation(v13[:, r0:r0 + RPC, 3:59],
                                     pv3[:, :, 3:59], AF.Identity,
                                     bias=bv_s[:])

            # ---- k/v padded borders hold conv(0)+bias = bias ----
            zt = pp.tile([CH, 1], BF16, tag="zt", name="zt")
            nc.vector.memset(zt[:], 0.0)
            sch.v += (58 + 1) / 0.96
            for plane, bias in ((k1, bk0_s[:]), (v1, bv_s[:])):
                p3 = plane[0:CH, :].rearrange("a (r c) -> a r c", c=WP)
                for view in (
                    plane[0:CH, 0: 3 * WP],                # rows 0-2
                    plane[0:CH, 59 * WP: NPAD],            # rows 59-61
                    p3[:, 3:59, 0:3],                      # left cols
                    p3[:, 3:59, 59:62],                    # right cols
                ):
                    sh = view.shape
                    if len(sh) == 2:
                        zin = zt[:, 0:1].broadcast_to((CH, sh[1]))
                    else:
                        zin = zt[:, 0:1].rearrange(
                            "a (x y) -> a x y", y=1).broadcast_to(
                            (CH, sh[1], sh[2]))
                    nc.scalar.activation(view, zin, AF.Identity, bias=bias)

            # ---- stage shifted B halves (+1 / +56) and k56/v56 A halves --
            for src, d1, d56 in ((k1, k1, k56), (v1, v1, v56)):
                nc.scalar.copy(d1[CH:128, 0:NPAD - 1], src[0:CH, 1:NPAD])
                nc.scalar.copy(d56[0:CH, :], src[0:CH, :])
                nc.scalar.copy(d56[CH:128, 0:NPAD - 56], src[0:CH, 56:NPAD])
                nc.vector.memset(d1[CH:128, NPAD - 1: NPAD], 0.0)
                nc.vector.memset(d56[CH:128, NPAD - 56: NPAD], 0.0)
                sch.v += (58 + 1) / 0.96 + (58 + 56) / 0.96

            # ---- QK: 25 pair products + bf16 add trees -> scores ----
            S_all = pp.tile([128, NPAIR * NSEG], F32, tag="Sall", name="Sall")
            qb3 = qb[:].rearrange("a (x y) -> a x y", y=W)

            def winv(t, i, j):
                t3 = t[:].rearrange("a (r c) -> a r c", c=WP)
                return t3[:, i:i + H, j:j + W]

            CHAIN_A = (_c_tt(NPOS) + _c_tt(NPOS // 2) + _c_tt(NPOS // 4)
                       + _c_tt(NPOS // 8) + _c_tt(NPOS // 16)
                       + _c_tt(NPOS // 32) + _c_tt(NPOS // 64, False))
            CHAIN_B = (CHAIN_A - _c_tt(NPOS // 2) - _c_tt(NPOS // 4)
                       - _c_tt(NPOS // 8))
            B_PE = 8 * MM(392)
            # n pairs offload tree l1-l3 to PE (8 strided-rhs identity
            # matmuls accumulate d-offsets 0,8,..,56 into PSUM 8-sums);
            # n solves the three-engine makespan balance
            N_PE_L1 = 9
            pe_set = set(round(i * (NPAIR - 1) / max(1, N_PE_L1 - 1))
                         for i in range(N_PE_L1)) if N_PE_L1 else set()
            NQ = NPAIR * NSEG
            PINNED_V = ((58 + 1) / 0.96
                        + 2 * ((58 + 1) / 0.96 + (58 + 56) / 0.96)
                        + 26 * (58 + NSEG) / 0.96
                        + 2 * _c_red(NQ)
                        + 4 * (58 + NSEG) / 0.96
                        + 7 * _c_tt(AVC))
            COSTS = ([CHAIN_B if q in pe_set else CHAIN_A
                      for q in range(NPAIR)]
                     + [_c_tt(NQ)]
                     + [_c_tt(NPOS)] * NPAIR
                     + [_c_tt(NPOS, False)])
            sch.make_plan(COSTS, PINNED_V)
            flip = 0
            for q in range(NPAIR):
                i, j, tk = PAIRS[q]
                ktile = k1 if tk == "k1" else k56
                use_pe = q in pe_set
                if use_pe:
                    sch.pe += B_PE
                    eng = sch.pick(CHAIN_B)
                else:
                    eng = sch.pick(CHAIN_A)
                prod = wp.tile([128, NPOS], BF16, tag="prod", name="prod",
                               bufs=3)
                eng.tensor_tensor(
                    out=prod[:].rearrange("a (x y) -> a x y", y=W),
                    in0=winv(ktile, i, j), in1=qb3, op=OP.mult)
                if use_pe:
                    # tree l1-l3 on PE: 8 strided-rhs identity matmuls
                    # accumulate d-offsets 0,8,..,56 into one PSUM bank of
                    # 8-sums; the idle ACT evicts them back to SBUF (DVE
                    # cannot read two PSUM operands in one op).
                    pt = psa[flip]
                    flip ^= 1
                    p3 = prod[:].rearrange("a (s d) -> a s d", d=SEG)
                    for off in range(0, SEG, 8):
                        nc.tensor.matmul(
                            pt[:, 0:NSEG * 8], lhsT=ident,
                            rhs=p3[:, :, off:off + 8],
                            start=(off == 0), stop=(off == SEG - 8))
                    t8 = wp.tile([128, NSEG * 8], BF16, tag="t8",
                                 name="t8", bufs=2)
                    nc.scalar.copy(
                        t8[:].rearrange("a (s d) -> a s d", d=8),
                        pt[:, 0:NSEG * 8].rearrange("a (s d) -> a s d",
                                                    d=8))
                    cur = t8
                    w = 8
                else:
                    cur = prod
                    w = SEG
                while w > 2:
                    w //= 2
                    t = wp.tile([128, NSEG * w], BF16, tag=f"t{w}",
                                name=f"t{w}", bufs=2)
                    c3 = cur[:].rearrange("a (s d) -> a s d", d=2 * w)
                    eng.tensor_tensor(
                        out=t[:].rearrange("a (s d) -> a s d", d=w),
                        in0=c3[:, :, 0:w], in1=c3[:, :, w:2 * w], op=OP.add)
                    cur = t
                sp = wp.tile([128, NSEG], F32, tag="sp", name="sp", bufs=2)
                c3 = cur[:].rearrange("a (s d) -> a s d", d=2)
                eng.tensor_tensor(
                    out=sp[:].rearrange("a (s o) -> a s o", o=1),
                    in0=c3[:, :, 0:1], in1=c3[:, :, 1:2], op=OP.add)
                # S = qsum*b49[p] + segsum  (rank-1 positional bias).
                # scalar_tensor_tensor fails Pool codegen: pin to DVE.
                sl = slice(NSEG * q, NSEG * (q + 1))
                sch.v += (58 + NSEG) / 0.96
                if q < NPAIR - 1:
                    nc.vector.scalar_tensor_tensor(
                        out=S_all[:, sl], in0=qsum_s[:],
                        scalar=b49_s[:, q:q + 1], in1=sp[:],
                        op0=OP.mult, op1=OP.add)
                else:
                    nc.vector.scalar_tensor_tensor(
                        out=S_all[0:CH, sl], in0=qsum_s[0:CH, :],
                        scalar=b49_s[0:CH, q:q + 1], in1=sp[0:CH, :],
                        op0=OP.mult, op1=OP.add)
            # mask the unused B half of the single shift p=48
            nc.vector.memset(S_all[CH:128, NSEG * 24: NSEG * 25], -1e30)
            sch.v += (58 + NSEG) / 0.96

            # ---- softmax over the 49 shifts (A/B halves + 25 columns) ----
            # reduce over the 25 pair-columns first ([128,49] per-half
            # stats), then combine halves via a tiny base-0 remap copy
            # (TensorTensor needs equal SBUF base partitions).
            mxq = pp.tile([128, NSEG], F32, tag="mxq", name="mxq")
            sch.v += _c_red(NQ)
            nc.vector.tensor_reduce(
                out=mxq[:],
                in_=S_all[:].rearrange("a (q s) -> a s q", s=NSEG),
                axis=AX.X, op=OP.max)
            mxb = pp.tile([CH, NSEG], F32, tag="mxb", name="mxb")
            nc.scalar.copy(mxb[:], mxq[CH:128, :])
            mxd = pp.tile([128, NSEG], F32, tag="mxd", name="mxd")
            sch.v += (58 + NSEG) / 0.96   # Pool rejects max: pin to DVE
            nc.vector.tensor_tensor(out=mxd[0:CH, :], in0=mxq[0:CH, :],
                                    in1=mxb[:], op=OP.max)
            nc.scalar.copy(mxd[CH:128, :], mxd[0:CH, :])
            sb = pp.tile([128, NQ], BF16, tag="sb", name="sb")
            eng = sch.pick(_c_tt(NQ))
            eng.tensor_tensor(
                out=sb[:].rearrange("a (q s) -> a q s", s=NSEG),
                in0=S_all[:].rearrange("a (q s) -> a q s", s=NSEG),
                in1=mxd[:].rearrange("a (o s) -> a o s", o=1).broadcast_to(
                    (128, NPAIR, NSEG)),
                op=OP.subtract)
            E = pp.tile([128, NQ], BF16, tag="E", name="E")
            nc.scalar.activation(E[:], sb[:], AF.Exp)
            dnq = pp.tile([128, NSEG], F32, tag="dnq", name="dnq")
            sch.v += _c_red(NQ)
            nc.vector.tensor_reduce(
                out=dnq[:],
                in_=E[:].rearrange("a (q s) -> a s q", s=NSEG),
                axis=AX.X, op=OP.add)
            dnb = pp.tile([CH, NSEG], F32, tag="dnb", name="dnb")
            nc.scalar.copy(dnb[:], dnq[CH:128, :])
            den = pp.tile([CH, NSEG], F32, tag="den", name="den")
            sch.v += (58 + NSEG) / 0.96
            nc.vector.tensor_tensor(out=den[:], in0=dnq[0:CH, :],
                                    in1=dnb[:], op=OP.add)
            rcp = pp.tile([128, NSEG], F32, tag="rcp", name="rcp")
            nc.vector.reciprocal(rcp[0:CH, :], den[:])
            sch.v += (58 + NSEG) / 0.96
            nc.scalar.copy(rcp[CH:128, :], rcp[0:CH, :])

            # ---- AV: weight-broadcast multiplies + PE accumulation ----
            for q in range(NPAIR):
                i, j, tk = PAIRS[q]
                vtile = v1 if tk == "k1" else v56
                eng = sch.pick(_c_tt(NPOS))
                vp = wp.tile([128, NPOS], BF16, tag="vp", name="vp", bufs=3)
                eng.tensor_tensor(
                    out=vp[:].rearrange("a (s d) -> a s d", d=SEG),
                    in0=winv(vtile, i, j),
                    in1=E[:, NSEG * q: NSEG * (q + 1)].rearrange(
                        "a (s o) -> a s o", o=1).broadcast_to(
                        (128, NSEG, SEG)),
                    op=OP.mult)
                for kch in range(NAV):
                    nc.tensor.matmul(
                        psa[kch][:, 0:AVC], lhsT=ident,
                        rhs=vp[:, AVC * kch: AVC * (kch + 1)],
                        start=(q == 0), stop=(q == NPAIR - 1))

            # ---- normalize, combine halves, store ----
            # full-width PSUM reads (gpsimd cannot read PSUM: pin to DVE),
            # then ACT remaps the B half to base 0 for the combine
            # (TensorTensor needs equal SBUF base partitions).
            fin = pp.tile([128, NPOS], BF16, tag="fin", name="fin")
            for kch in range(NAV):
                csl = slice(AVC * kch, AVC * (kch + 1))
                sch.v += _c_tt(AVC)
                nc.vector.tensor_tensor(
                    out=fin[:, csl].rearrange("a (s d) -> a s d", d=SEG),
                    in0=psa[kch][:, 0:AVC].rearrange(
                        "a (s d) -> a s d", d=SEG),
                    in1=rcp[:, 7 * kch: 7 * (kch + 1)].rearrange(
                        "a (s o) -> a s o", o=1).broadcast_to(
                        (128, 7, SEG)),
                    op=OP.mult)
            fb0 = pp.tile([CH, NPOS], BF16, tag="fb0", name="fb0")
            nc.scalar.copy(fb0[:], fin[CH:128, :])
            fout = pp.tile([CH, NPOS], F32, tag="fout", name="fout")
            eng = sch.pick(_c_tt(NPOS, False))
            eng.tensor_tensor(out=fout[:], in0=fin[0:CH, :], in1=fb0[:],
                              op=OP.add)
            nc.sync.dma_start(out_d[:], fout[:])
    return nc


import json


def _legalize_waits(bir_bytes):
    """Walrus codegen rejects >1 semaphore wait per instruction; hoist the
    extras onto NoOps (same engine, immediately before) so every
    instruction carries at most one wait."""
    bir = json.loads(bir_bytes)
    ctr = [0]

    def fix_block(instructions):
        out = []
        for ins in instructions:
            si = ins.get("sync_info")
            if si:
                w = si.get("on_wait") or []
                if len(w) > 1:
                    for extra in w[:-1]:
                        ctr[0] += 1
                        out.append({
                            "debug": ins.get("debug", 0),
                            "engine": ins["engine"],
                            "ins": [], "outs": [],
                            "name": f"I-lw{ctr[0]}",
                            "opcode": "NoOp",
                            "sync_info": {"on_wait": [extra],
                                          "on_update": []},
                        })
                    si["on_wait"] = [w[-1]]
            out.append(ins)
        instructions[:] = out

    def walk(o):
        if isinstance(o, dict):
            if "instructions" in o:
                fix_block(o["instructions"])
            for v in o.values():
                walk(v)
        elif isinstance(o, list):
            for v in o:
                walk(v)

    walk(bir)
    return json.dumps(bir).encode()


_NC_CACHE = {}


def kernel(x, q_w, q_b, k_w, k_b, v_w, v_b, h_pos, w_pos):
    x = np.asarray(x, np.float64)
    xp32 = np.pad(x[0], ((0, 0), (3, 3), (3, 3))).reshape(C, NPAD).astype(
        np.float32)
    bias49 = (np.asarray(h_pos, np.float64).sum(0)
              + np.asarray(w_pos, np.float64).sum(0)).reshape(NSH)
    # per-pair positional-bias column: rows 0-63 = bias49[2q], 64-127 =
    # bias49[2q+1] (0 for the unused B half of the last single)
    b49cols = np.zeros((128, NPAIR), np.float64)
    for q in range(NPAIR):
        b49cols[0:CH, q] = bias49[2 * q]
        if q < NPAIR - 1:
            b49cols[CH:128, q] = bias49[2 * q + 1]
    b49cols = np.ascontiguousarray(b49cols.astype(np.float32))

    # exact host qsum: sum_d q[c,seg] = q_w[c,:] @ (seg-sums of x) + 64*q_b
    xs = x[0].reshape(C, NSEG, SEG).sum(-1)              # [512, 49] f64
    qsum_all = (np.asarray(q_w, np.float64) @ xs
                + 64.0 * np.asarray(q_b, np.float64)[:, None])  # [512, 49]

    eye_u16 = (np.eye(128, dtype=np.uint16) * 0x3F80)    # bf16 1.0
    identp = np.ascontiguousarray(eye_u16.view(np.float32))

    in_maps = []
    chan_lists = []
    for r in range(N_CORES):
        chans = np.array([64 * h + 8 * r + t for h in range(8)
                          for t in range(8)])
        chan_lists.append(chans)
        wq = np.asarray(q_w, np.float32)[chans, :]
        wk = np.asarray(k_w, np.float32)[chans, :]
        wv = np.asarray(v_w, np.float32)[chans, :]
        wTl = np.ascontiguousarray(
            np.concatenate([wq.T, wk.T, wv.T], axis=1))
        bqk = np.concatenate([np.asarray(q_b, np.float32)[chans],
                              np.asarray(k_b, np.float32)[chans]])
        qsd = np.ascontiguousarray(
            np.tile(qsum_all[chans].astype(np.float32), (2, 1)))
        in_maps.append({
            "xp": xp32,
            "wT": wTl,
            "bqk": np.ascontiguousarray(bqk[:, None]),
            "bv": np.ascontiguousarray(
                np.asarray(v_b, np.float32)[chans][:, None]),
            "bk0": np.ascontiguousarray(
                np.asarray(k_b, np.float32)[chans][:, None]),
            "b49c": b49cols,
            "qsumd": qsd,
            "identp": identp,
        })

    if "nc" not in _NC_CACHE:
        nc = _build_nc()
        legal = _legalize_waits(nc.to_json_bytes())
        nc.to_json_bytes = lambda: legal
        _NC_CACHE["nc"] = nc
    res = run_bass_kernel_spmd(_NC_CACHE["nc"], in_maps,
                               list(range(N_CORES)))
    _NC_CACHE["last_results"] = res

    out = np.empty((C, NPOS), np.float32)
    for r in range(N_CORES):
        out[chan_lists[r], :] = np.asarray(res.results[r]["out"])
    return out.reshape(1, C, H, W)


if __name__ == "__main__":
    _build_nc()
    print("build OK")


# revision 49
# speedup vs baseline: 1.0009x; 1.0009x over previous
"""Trainium2 Bass kernel for nn_Attention_layer_12249246728743.

Reference structure (after untangling the C-order reshape): per channel c
of 512, the 3136 raster positions split into 49 segments of 64
consecutive positions; each segment attends over a 7x7 shifted window of
its OWN channel plane (depthwise local attention):

  scores[c,s,p=(i,j)] = sum_d q[c,64s+d] * k[c, win(64s+d, i, j)]
                        + (sum_d q[c,64s+d]) * bias49[p]
  w = softmax_p(scores);  out[c,64s+d] = sum_p w[c,s,p] * v[c, win(...)]

with q/k/v = 1x1 convs of x (k, v on the zero-padded 62x62 domain).

Sharding: channel-parallel across 8 cores (64 channels each); every
attention segment is core-local: no halo, no collectives.

Layout: "pair-packed" attention - partitions = 64 channels x 2
shift-pair halves, free dim = the full 3136-position raster. The B half
holds k/v planes pre-shifted by +1 (or +56 for row-wrapping pairs), so
ONE tensor op computes two of the 49 window shifts at once (25 ops
instead of 49, no half-image padding waste). QK segment sums use
contiguous-half bf16 add trees (2x DVE mode) + a small TensorReduce
instead of full-width fp32 reduces. AV weights enter the multiply as
stride-0 broadcast APs (free-size-matched zip with the window AP), and
the 25 AV product tiles are accumulated on the otherwise-idle PE via
identity matmuls into PSUM. qsum (rank-1 bias term) is exact host-side
algebra (q_w @ segment-sums-of-x), mirroring the host-collapsed bias49.
Work is split between DVE and GPSIMD by a greedy makespan balancer.
"""

import numpy as np

import concourse.bass as bass
import concourse.mybir as mybir
import concourse.tile as tile
from concourse.bass_utils import run_bass_kernel_spmd

F32 = mybir.dt.float32
BF16 = mybir.dt.bfloat16
AX = mybir.AxisListType
OP = mybir.AluOpType
AF = mybir.ActivationFunctionType

N_CORES = 8
C = 512
H = W = 56
HP = WP = 62          # padded spatial
NPOS = H * W          # 3136
NPAD = HP * WP        # 3844
K = 7
NSH = K * K           # 49 shifts
SEG = 64              # positions per attention segment
NSEG = NPOS // SEG    # 49 segments per channel
CH = 64               # channels per core
NPAIR = 25            # 24 shift pairs + 1 single (p=48)

# shift pairs (pA=2t, pB=2t+1): B half of the k/v tiles is pre-shifted by
# +1 (same-row j->j+1) or +56 (row wrap (i,6)->(i+1,0)). Window slice is
# always pA's (i,j) clamped to j<=6.
def _pair_table():
    pairs = []
    for t in range(24):
        pA = 2 * t
        i, j = divmod(pA, K)
        if j < K - 1:
            pairs.append((i, j, "k1"))     # B = (i, j+1) via +1 tile
        else:
            pairs.append((i, j, "k56"))    # B = (i+1, 0) via +56 tile
    pairs.append((6, 6, "k1"))             # p=48 single; B half masked
    return pairs


PAIRS = _pair_table()

# --- naive op-cost mirror (documented TRN2 formulas) for the balancer ---
def _c_tt(fd, bf16_out=True):
    return (58 + (fd / 2 if bf16_out else fd)) / 0.96


def _c_red(fd_in):
    return (58 + fd_in) / 0.96


class _Sched:
    """DVE/GPSIMD makespan balancer (Pool runs at 2x cost).

    Costs are static, so assignment is planned offline: LPT over the
    descending-sorted schedulable costs against the DVE-pinned base load,
    consumed in emission order by pick().
    """

    def __init__(self, nc, pe_base):
        self.nc = nc
        self.v = 0.0
        self.g = 0.0
        self.pe = pe_base
        self.plan = []

    def make_plan(self, costs, pinned_v):
        v, g = pinned_v, 0.0
        asg = [None] * len(costs)
        for i in sorted(range(len(costs)), key=lambda i: -costs[i]):
            c = costs[i]
            if max(v + c, 2 * g) <= max(v, 2 * (g + c)):
                v += c
                asg[i] = "v"
            else:
                g += c
                asg[i] = "g"
        self.plan = asg[::-1]          # consume via pop()

    def pick(self, cost):
        e = self.plan.pop() if self.plan else (
            "v" if max(self.v + cost, self.g) <= max(self.v,
                                                     self.g + 2 * cost)
            else "g")
        if e == "v":
            self.v += cost
            return self.nc.vector
        self.g += 2 * cost
        return self.nc.gpsimd




def _build_nc():
    nc = bass.Bass()

    xp = nc.declare_dram_parameter("xp", [C, NPAD], F32, isOutput=False)
    wT = nc.declare_dram_parameter("wT", [C, 3 * CH], F32, isOutput=False)
    bqk = nc.declare_dram_parameter("bqk", [2 * CH, 1], F32, isOutput=False)
    bv = nc.declare_dram_parameter("bv", [CH, 1], F32, isOutput=False)
    bk0 = nc.declare_dram_parameter("bk0", [CH, 1], F32, isOutput=False)
    b49c = nc.declare_dram_parameter("b49c", [128, NPAIR], F32,
                                     isOutput=False)
    qsumd = nc.declare_dram_parameter("qsumd", [128, NSEG], F32,
                                      isOutput=False)
    identp = nc.declare_dram_parameter("identp", [128, 64], F32,
                                       isOutput=False)
    out_d = nc.declare_dram_parameter("out", [CH, NPOS], F32, isOutput=True)

    RPC = 8                      # rows per conv chunk
    NCH = RPC * WP               # 496 conv cols per chunk
    NCHI = RPC * W               # 448 interior positions per chunk
    NCK = 7                      # conv chunks (7*8 = 56 rows)
    AVC = 448                    # AV psum chunk (7 segments)
    NAV = 7                      # AV chunks

    with tile.TileContext(nc) as tc:
        with (
            tc.tile_pool(name="persist", bufs=1) as pp,
            tc.tile_pool(name="work", bufs=2) as wp,
            tc.tile_pool(name="psum", bufs=1, space="PSUM") as psp,
        ):
            MM = lambda n: max(60, 6 + n) / 2.4 + 128 / 1.2
            PE_BASE = 56 * MM(496) + MM(1) + 25 * 7 * MM(448)
            sch = _Sched(nc, PE_BASE)

            # ---- loads ----
            xt_all = pp.tile([128, 4 * NPAD], F32, tag="xall", name="xall")
            wt_all = pp.tile([128, 4 * 3 * CH], F32, tag="wall", name="wall")
            for s0 in range(0, NPAD, 1024):
                sn = min(1024, NPAD - s0)
                nc.sync.dma_start(
                    xt_all[:].rearrange("p (k n) -> p k n", k=4)[:, :, s0:s0 + sn],
                    xp[:].rearrange("(k p) n -> p k n", p=128)[:, :, s0:s0 + sn])
            nc.sync.dma_start(
                wt_all[:].rearrange("p (k n) -> p k n", k=4),
                wT[:].rearrange("(k p) n -> p k n", p=128))
            xt = [xt_all[:].rearrange("p (k n) -> p k n", k=4)[:, kt, :]
                  for kt in range(4)]
            wt = [wt_all[:].rearrange("p (k n) -> p k n", k=4)[:, kt, :]
                  for kt in range(4)]
            bqk_s = pp.tile([128, 1], F32, tag="bqk", name="bqk")
            bv_s = pp.tile([CH, 1], F32, tag="bv", name="bv")
            bk0_s = pp.tile([CH, 1], F32, tag="bk0", name="bk0")
            nc.sync.dma_start(bk0_s[:], bk0[:])
            b49_s = pp.tile([128, NPAIR], F32, tag="b49", name="b49")
            qsum_s = pp.tile([128, NSEG], F32, tag="qsum", name="qsum")
            idf_s = pp.tile([128, 64], F32, tag="idf", name="idf")
            nc.sync.dma_start(bqk_s[:], bqk[:])
            nc.sync.dma_start(bv_s[:], bv[:])
            nc.sync.dma_start(b49_s[:], b49c[:])
            nc.sync.dma_start(qsum_s[:], qsumd[:])
            nc.sync.dma_start(idf_s[:], identp[:])
            ident = idf_s[:].bitcast(BF16)          # [128, 128] bf16 identity

            # ---- attention-layout staging tiles ----
            qb = pp.tile([128, NPOS], BF16, tag="qb", name="qb")
            k1 = pp.tile([128, NPAD], BF16, tag="k1", name="k1")
            k56 = pp.tile([128, NPAD], BF16, tag="k56", name="k56")
            v1 = pp.tile([128, NPAD], BF16, tag="v1", name="v1")
            v56 = pp.tile([128, NPAD], BF16, tag="v56", name="v56")

            # conv / QK-tree-l1 / AV shared PSUM accumulators (8 banks)
            psa = [psp.tile([128, 512], F32, tag=f"psa{j}", name=f"psa{j}")
                   for j in range(8)]

            # PE pre-touch (keeps every real Matmult at <=1 sem wait for
            # walrus S3_LW codegen).
            nc.tensor.matmul(psa[7][0:1, 0:1], lhsT=xt_all[0:1, 0:1],
                             rhs=xt_all[0:1, 0:1], start=True, stop=True)

            # ---- 1x1 convs on interior rows (rows 3..58 of padded) ----
            for ci in range(NCK):
                r0 = 3 + RPC * ci
                pq = psa[(2 * ci) % 6]
                pv = psa[(2 * ci) % 6 + 1]
                for kt in range(4):
                    rhs = xt[kt][:, WP * r0: WP * r0 + NCH]
                    nc.tensor.matmul(pq[:, 0:NCH], lhsT=wt[kt][:, 0:128],
                                     rhs=rhs, start=(kt == 0), stop=(kt == 3))
                    nc.tensor.matmul(pv[0:CH, 0:NCH],
                                     lhsT=wt[kt][:, 128:192],
                                     rhs=rhs, start=(kt == 0), stop=(kt == 3))
                pq3 = pq[:, 0:NCH].rearrange("a (r c) -> a r c", c=WP)
                pv3 = pv[0:CH, 0:NCH].rearrange("a (r c) -> a r c", c=WP)
                qbv = qb[:, NCHI * ci: NCHI * (ci + 1)].rearrange(
                    "a (r c) -> a r c", c=W)
                nc.scalar.activation(qbv[0:CH], pq3[0:CH, :, 3:59],
                                     AF.Identity, bias=bqk_s[0:CH, :])
                nc.scalar.activation(qbv[CH:128], pq3[0:CH, :, 3:59],
                                     AF.Identity, bias=bqk_s[0:CH, :])
                k13 = k1[0:CH, :].rearrange("a (r c) -> a r c", c=WP)
                v13 = v1[0:CH, :].rearrange("a (r c) -> a r c", c=WP)
                nc.scalar.activation(k13[:, r0:r0 + RPC, 3:59],
                                     pq3[CH:128, :, 3:59], AF.Identity,
                                     bias=bqk_s[CH:128, :])
                nc.scalar.activation(v13[:, r0:r0 + RPC, 3:59],
                                     pv3[:, :, 3:59], AF.Identity,
                                     bias=bv_s[:])

            # ---- k/v padded borders hold conv(0)+bias = bias ----
            zt = pp.tile([CH, 1], BF16, tag="zt", name="zt")
            nc.vector.memset(zt[:], 0.0)
            sch.v += (58 + 1) / 0.96
            for plane, bias in ((k1, bk0_s[:]), (v1, bv_s[:])):
                p3 = plane[0:CH, :].rearrange("a (r c) -> a r c", c=WP)
                for view in (
                    plane[0:CH, 0: 3 * WP],                # rows 0-2
                    plane[0:CH, 59 * WP: NPAD],            # rows 59-61
                    p3[:, 3:59, 0:3],                      # left cols
                    p3[:, 3:59, 59:62],                    # right cols
                ):
                    sh = view.shape
                    if len(sh) == 2:
                        zin = zt[:, 0:1].broadcast_to((CH, sh[1]))
                    else:
                        zin = zt[:, 0:1].rearrange(
                            "a (x y) -> a x y", y=1).broadcast_to(
                            (CH, sh[1], sh[2]))
                    nc.scalar.activation(view, zin, AF.Identity, bias=bias)

            # ---- stage shifted B halves (+1 / +56) and k56/v56 A halves --
            for src, d1, d56 in ((k1, k1, k56), (v1, v1, v56)):
                nc.scalar.copy(d1[CH:128, 0:NPAD - 1], src[0:CH, 1:NPAD])
                nc.scalar.copy(d56[0:CH, :], src[0:CH, :])
                nc.scalar.copy(d56[CH:128, 0:NPAD - 56], src[0:CH, 56:NPAD])
                nc.vector.memset(d1[CH:128, NPAD - 1: NPAD], 0.0)
                nc.vector.memset(d56[CH:128, NPAD - 56: NPAD], 0.0)
                sch.v += (58 + 1) / 0.96 + (58 + 56) / 0.96

            # ---- QK: 25 pair products + bf16 add trees -> scores ----
            S_all = pp.tile([128, NPAIR * NSEG], F32, tag="Sall", name="Sall")
            qb3 = qb[:].rearrange("a (x y) -> a x y", y=W)

            def winv(t, i, j):
                t3 = t[:].rearrange("a (r c) -> a r c", c=WP)
                return t3[:, i:i + H, j:j + W]

            CHAIN_A = (_c_tt(NPOS) + _c_tt(NPOS // 2) + _c_tt(NPOS // 4)
                       + _c_tt(NPOS // 8) + _c_tt(NPOS // 16)
                       + _c_tt(NPOS // 32) + _c_tt(NPOS // 64, False))
            CHAIN_B = (CHAIN_A - _c_tt(NPOS // 2) - _c_tt(NPOS // 4)
                       - _c_tt(NPOS // 8))
            B_PE = 8 * MM(392)
            # n pairs offload tree l1-l3 to PE (8 strided-rhs identity
            # matmuls accumulate d-offsets 0,8,..,56 into PSUM 8-sums);
            # n solves the three-engine makespan balance
            N_PE_L1 = 8
            pe_set = set(round(i * (NPAIR - 1) / max(1, N_PE_L1 - 1))
                         for i in range(N_PE_L1)) if N_PE_L1 else set()
            NQ = NPAIR * NSEG
            PINNED_V = ((58 + 1) / 0.96
                        + 2 * ((58 + 1) / 0.96 + (58 + 56) / 0.96)
                        + 26 * (58 + NSEG) / 0.96
                        + 2 * _c_red(NQ)
                        + 4 * (58 + NSEG) / 0.96)
            COSTS = ([CHAIN_B if q in pe_set else CHAIN_A
                      for q in range(NPAIR)]
                     + [_c_tt(NQ), _c_tt(NQ)]
                     + [_c_tt(NPOS)] * NPAIR
                     + [_c_tt(NPOS, False)])
            sch.make_plan(COSTS, PINNED_V)
            flip = 0
            for q in range(NPAIR):
                i, j, tk = PAIRS[q]
                ktile = k1 if tk == "k1" else k56
                use_pe = q in pe_set
                if use_pe:
                    sch.pe += B_PE
                    eng = sch.pick(CHAIN_B)
                else:
                    eng = sch.pick(CHAIN_A)
                prod = wp.tile([128, NPOS], BF16, tag="prod", name="prod",
                               bufs=3)
                eng.tensor_tensor(
                    out=prod[:].rearrange("a (x y) -> a x y", y=W),
                    in0=winv(ktile, i, j), in1=qb3, op=OP.mult)
                if use_pe:
                    # tree l1-l3 on PE: 8 strided-rhs identity matmuls
                    # accumulate d-offsets 0,8,..,56 into one PSUM bank of
                    # 8-sums; the idle ACT evicts them back to SBUF (DVE
                    # cannot read two PSUM operands in one op).
                    pt = psa[flip]
                    flip ^= 1
                    p3 = prod[:].rearrange("a (s d) -> a s d", d=SEG)
                    for off in range(0, SEG, 8):
                        nc.tensor.matmul(
                            pt[:, 0:NSEG * 8], lhsT=ident,
                            rhs=p3[:, :, off:off + 8],
                            start=(off == 0), stop=(off == SEG - 8))
                    t8 = wp.tile([128, NSEG * 8], BF16, tag="t8",
                                 name="t8", bufs=2)
                    nc.scalar.copy(
                        t8[:].rearrange("a (s d) -> a s d", d=8),
                        pt[:, 0:NSEG * 8].rearrange("a (s d) -> a s d",
                                                    d=8))
                    cur = t8
                    w = 8
                else:
                    cur = prod
                    w = SEG
                while w > 2:
                    w //= 2
                    t = wp.tile([128, NSEG * w], BF16, tag=f"t{w}",
                                name=f"t{w}", bufs=2)
                    c3 = cur[:].rearrange("a (s d) -> a s d", d=2 * w)
                    eng.tensor_tensor(
                        out=t[:].rearrange("a (s d) -> a s d", d=w),
                        in0=c3[:, :, 0:w], in1=c3[:, :, w:2 * w], op=OP.add)
                    cur = t
                sp = wp.tile([128, NSEG], F32, tag="sp", name="sp", bufs=2)
                c3 = cur[:].rearrange("a (s d) -> a s d", d=2)
                eng.tensor_tensor(
                    out=sp[:].rearrange("a (s o) -> a s o", o=1),
                    in0=c3[:, :, 0:1], in1=c3[:, :, 1:2], op=OP.add)
                # S = qsum*b49[p] + segsum  (rank-1 positional bias).
                # scalar_tensor_tensor fails Pool codegen: pin to DVE.
                sl = slice(NSEG * q, NSEG * (q + 1))
                sch.v += (58 + NSEG) / 0.96
                if q < NPAIR - 1:
                    nc.vector.scalar_tensor_tensor(
                        out=S_all[:, sl], in0=qsum_s[:],
                        scalar=b49_s[:, q:q + 1], in1=sp[:],
                        op0=OP.mult, op1=OP.add)
                else:
                    nc.vector.scalar_tensor_tensor(
                        out=S_all[0:CH, sl], in0=qsum_s[0:CH, :],
                        scalar=b49_s[0:CH, q:q + 1], in1=sp[0:CH, :],
                        op0=OP.mult, op1=OP.add)
            # mask the unused B half of the single shift p=48
            nc.vector.memset(S_all[CH:128, NSEG * 24: NSEG * 25], -1e30)
            sch.v += (58 + NSEG) / 0.96

            # ---- softmax over the 49 shifts (A/B halves + 25 columns) ----
            # reduce over the 25 pair-columns first ([128,49] per-half
            # stats), then combine halves via a tiny base-0 remap copy
            # (TensorTensor needs equal SBUF base partitions).
            mxq = pp.tile([128, NSEG], F32, tag="mxq", name="mxq")
            sch.v += _c_red(NQ)
            nc.vector.tensor_reduce(
                out=mxq[:],
                in_=S_all[:].rearrange("a (q s) -> a s q", s=NSEG),
                axis=AX.X, op=OP.max)
            mxb = pp.tile([CH, NSEG], F32, tag="mxb", name="mxb")
            nc.scalar.copy(mxb[:], mxq[CH:128, :])
            mxd = pp.tile([128, NSEG], F32, tag="mxd", name="mxd")
            sch.v += (58 + NSEG) / 0.96   # Pool rejects max: pin to DVE
            nc.vector.tensor_tensor(out=mxd[0:CH, :], in0=mxq[0:CH, :],
                                    in1=mxb[:], op=OP.max)
            nc.scalar.copy(mxd[CH:128, :], mxd[0:CH, :])
            sb = pp.tile([128, NQ], BF16, tag="sb", name="sb")
            eng = sch.pick(_c_tt(NQ))
            eng.tensor_tensor(
                out=sb[:].rearrange("a (q s) -> a q s", s=NSEG),
                in0=S_all[:].rearrange("a (q s) -> a q s", s=NSEG),
                in1=mxd[:].rearrange("a (o s) -> a o s", o=1).broadcast_to(
                    (128, NPAIR, NSEG)),
                op=OP.subtract)
            E = pp.tile([128, NQ], BF16, tag="E", name="E")
            nc.scalar.activation(E[:], sb[:], AF.Exp)
            dnq = pp.tile([128, NSEG], F32, tag="dnq", name="dnq")
            sch.v += _c_red(NQ)
            nc.vector.tensor_reduce(
                out=dnq[:],
                in_=E[:].rearrange("a (q s) -> a s q", s=NSEG),
                axis=AX.X, op=OP.add)
            dnb = pp.tile([CH, NSEG], F32, tag="dnb", name="dnb")
            nc.scalar.copy(dnb[:], dnq[CH:128, :])
            den = pp.tile([CH, NSEG], F32, tag="den", name="den")
            sch.v += (58 + NSEG) / 0.96
            nc.vector.tensor_tensor(out=den[:], in0=dnq[0:CH, :],
                                    in1=dnb[:], op=OP.add)
            rcp = pp.tile([128, NSEG], F32, tag="rcp", name="rcp")
            nc.vector.reciprocal(rcp[0:CH, :], den[:])
            sch.v += (58 + NSEG) / 0.96
            nc.scalar.copy(rcp[CH:128, :], rcp[0:CH, :])
            # pre-normalize the weights so PSUM accumulates the final
            # output directly: the post-AV pass becomes plain ACT evicts
            EN = pp.tile([128, NQ], BF16, tag="EN", name="EN")
            eng = sch.pick(_c_tt(NQ))
            eng.tensor_tensor(
                out=EN[:].rearrange("a (q s) -> a q s", s=NSEG),
                in0=E[:].rearrange("a (q s) -> a q s", s=NSEG),
                in1=rcp[:].rearrange("a (o s) -> a o s", o=1).broadcast_to(
                    (128, NPAIR, NSEG)),
                op=OP.mult)

            # ---- AV: weight-broadcast multiplies + PE accumulation ----
            for q in range(NPAIR):
                i, j, tk = PAIRS[q]
                vtile = v1 if tk == "k1" else v56
                eng = sch.pick(_c_tt(NPOS))
                vp = wp.tile([128, NPOS], BF16, tag="vp", name="vp", bufs=3)
                eng.tensor_tensor(
                    out=vp[:].rearrange("a (s d) -> a s d", d=SEG),
                    in0=winv(vtile, i, j),
                    in1=EN[:, NSEG * q: NSEG * (q + 1)].rearrange(
                        "a (s o) -> a s o", o=1).broadcast_to(
                        (128, NSEG, SEG)),
                    op=OP.mult)
                for kch in range(NAV):
                    nc.tensor.matmul(
                        psa[kch][:, 0:AVC], lhsT=ident,
                        rhs=vp[:, AVC * kch: AVC * (kch + 1)],
                        start=(q == 0), stop=(q == NPAIR - 1))

            # ---- combine halves, store ----
            # idle ACT evicts the normalized PSUM sums; B half remapped to
            # base 0 for the combine (TensorTensor needs equal SBUF bases).
            fin = pp.tile([128, NPOS], BF16, tag="fin", name="fin")
            for kch in range(NAV):
                nc.scalar.copy(fin[:, AVC * kch: AVC * (kch + 1)],
                               psa[kch][:, 0:AVC])
            fb0 = pp.tile([CH, NPOS], BF16, tag="fb0", name="fb0")
            nc.scalar.copy(fb0[:], fin[CH:128, :])
            fout = pp.tile([CH, NPOS], F32, tag="fout", name="fout")
            eng = sch.pick(_c_tt(NPOS, False))
            eng.tensor_tensor(out=fout[:], in0=fin[0:CH, :], in1=fb0[:],
                              op=OP.add)
            nc.sync.dma_start(out_d[:], fout[:])
    return nc


import json


def _legalize_waits(bir_bytes):
    """Walrus codegen rejects >1 semaphore wait per instruction; hoist the
    extras onto NoOps (same engine, immediately before) so every
    instruction carries at most one wait."""
    bir = json.loads(bir_bytes)
    ctr = [0]

    def fix_block(instructions):
        out = []
        for ins in instructions:
            si = ins.get("sync_info")
            if si:
                w = si.get("on_wait") or []
                if len(w) > 1:
                    for extra in w[:-1]:
                        ctr[0] += 1
                        out.append({
                            "debug": ins.get("debug", 0),
                            "engine": ins["engine"],
                            "ins": [], "outs": [],
                            "name": f"I-lw{ctr[0]}",
                            "opcode": "NoOp",
                            "sync_info": {"on_wait": [extra],
                                          "on_update": []},
                        })
                    si["on_wait"] = [w[-1]]
            out.append(ins)
        instructions[:] = out

    def walk(o):
        if isinstance(o, dict):
            if "instructions" in o:
                fix_block(o["instructions"])
            for v in o.values():
                walk(v)
        elif isinstance(o, list):
            for v in o:
                walk(v)

    walk(bir)
    return json.dumps(bir).encode()


_NC_CACHE = {}


def kernel(x, q_w, q_b, k_w, k_b, v_w, v_b, h_pos, w_pos):
    x = np.asarray(x, np.float64)
    xp32 = np.pad(x[0], ((0, 0), (3, 3), (3, 3))).reshape(C, NPAD).astype(
        np.float32)
    bias49 = (np.asarray(h_pos, np.float64).sum(0)
              + np.asarray(w_pos, np.float64).sum(0)).reshape(NSH)
    # per-pair positional-bias column: rows 0-63 = bias49[2q], 64-127 =
    # bias49[2q+1] (0 for the unused B half of the last single)
    b49cols = np.zeros((128, NPAIR), np.float64)
    for q in range(NPAIR):
        b49cols[0:CH, q] = bias49[2 * q]
        if q < NPAIR - 1:
            b49cols[CH:128, q] = bias49[2 * q + 1]
    b49cols = np.ascontiguousarray(b49cols.astype(np.float32))

    # exact host qsum: sum_d q[c,seg] = q_w[c,:] @ (seg-sums of x) + 64*q_b
    xs = x[0].reshape(C, NSEG, SEG).sum(-1)              # [512, 49] f64
    qsum_all = (np.asarray(q_w, np.float64) @ xs
                + 64.0 * np.asarray(q_b, np.float64)[:, None])  # [512, 49]

    eye_u16 = (np.eye(128, dtype=np.uint16) * 0x3F80)    # bf16 1.0
    identp = np.ascontiguousarray(eye_u16.view(np.float32))

    in_maps = []
    chan_lists = []
    for r in range(N_CORES):
        chans = np.array([64 * h + 8 * r + t for h in range(8)
                          for t in range(8)])
        chan_lists.append(chans)
        wq = np.asarray(q_w, np.float32)[chans, :]
        wk = np.asarray(k_w, np.float32)[chans, :]
        wv = np.asarray(v_w, np.float32)[chans, :]
        wTl = np.ascontiguousarray(
            np.concatenate([wq.T, wk.T, wv.T], axis=1))
        bqk = np.concatenate([np.asarray(q_b, np.float32)[chans],
                              np.asarray(k_b, np.float32)[chans]])
        qsd = np.ascontiguousarray(
            np.tile(qsum_all[chans].astype(np.float32), (2, 1)))
        in_maps.append({
            "xp": xp32,
            "wT": wTl,
            "bqk": np.ascontiguousarray(bqk[:, None]),
            "bv": np.ascontiguousarray(
                np.asarray(v_b, np.float32)[chans][:, None]),
            "bk0": np.ascontiguousarray(
                np.asarray(k_b, np.float32)[chans][:, None]),
            "b49c": b49cols,
            "qsumd": qsd,
            "identp": identp,
        })

    if "nc" not in _NC_CACHE:
        nc = _build_nc()
        legal = _legalize_waits(nc.to_json_bytes())
        nc.to_json_bytes = lambda: legal
        _NC_CACHE["nc"] = nc
    res = run_bass_kernel_spmd(_NC_CACHE["nc"], in_maps,
                               list(range(N_CORES)))
    _NC_CACHE["last_results"] = res

    out = np.empty((C, NPOS), np.float32)
    for r in range(N_CORES):
        out[chan_lists[r], :] = np.asarray(res.results[r]["out"])
    return out.reshape(1, C, H, W)


if __name__ == "__main__":
    _build_nc()
    print("build OK")
